# revision 11
# baseline (speedup 1.0000x reference)
"""Trainium2 Bass kernel for windowed multi-head attention (nn_AttentionWindow).

Reference computation (B=64, N=197, DIM=768, H=12, HD=64):
    qkv  = x @ qkv_w.T + [q_bias, 0, v_bias]
    q, k, v = split(qkv);  q *= HD**-0.5
    attn = softmax(q @ k.T + rpb_table[rel_index])
    out  = (attn @ v) @ proj_w.T + proj_b
Sharding: data-parallel over batch across 8 NeuronCores (8 batches/core).

Per-core design (bf16 matmuls on TensorE, fp32 PSUM accumulation), fully
software-pipelined over 2-batch groups so TensorE never idles (keeps the
PE_HAM clock gate at 2.4 GHz):
  - x pre-transposed on host to xT [768, 1576] bf16, resident in SBUF.
    q,k computed feature-major into a resident [128, 12, 1576] bf16 tile,
    v token-major per batch ([tokens, channels] bf16, 128+69 row chunks).
  - Attention per (head-pair, batch) item: S^T[j,i] = k_h[:,j]^T q_h into
    per-head PSUM banks [128,512] (j1 @0:197, j2 @256:453), heads of the
    pair PE row-packed (concurrent). Softmax WITHOUT max subtraction
    (scores O(1): q pre-scaled by 1/8): one strided exp per head on
    ScalarE -> per-head P^T tile [128, 2, 197] bf16 (j2 rows 69:128 are
    don't-care), one contiguous DVE multiply per head against the
    precomputed exp(bias) table ebh: softmax(S+B) = expS*expB / sums.
  - O^T col-packed head pair at cols 0:197 of one PSUM bank, softmax
    denominators via ones[*,64] matmuls at cols 256:453 of the same bank
    (h0 on partitions 0:64, h1 on 64:128) -> single-op
    reciprocal_approx_fast + single-op normalizing multiply (fused into
    the PSUM->SBUF copy assembling proj's rhs op_s [128, 6, 1576] bf16).
  - The qkv/v GEMM chains of group g+1 and the projection of group g are
    interleaved between group g's attention items as dependency-free PE
    filler: attention's softmax latency chain hides under them.
  - DMA order: xb/qkw rows first (unblock the first GEMMs), vw, bias
    tables, then pw (needed only ~60us in).
"""
import sys
import functools

sys.path.insert(0, "/opt/trn_rl_repo")

import numpy as np
import ml_dtypes

import concourse.bass as bass  # noqa: E402
import concourse.bacc as bacc  # noqa: E402
import concourse.mybir as mybir  # noqa: E402
from concourse.tile import TileContext  # noqa: E402
from concourse.bass_utils import run_bass_kernel_spmd  # noqa: E402

F32 = mybir.dt.float32
BF16 = mybir.dt.bfloat16

NCORES = 8
B, NT, DIM = 64, 197, 768
H, HD = 12, 64
SCALE = HD ** -0.5  # 0.125, exact power of two -> folded into q weights
BPC = B // NCORES   # 8 batches per core
TOK = BPC * NT      # 1576 tokens per core
KC = DIM // 128     # 6
NT2 = NT - 128      # 69 (second token chunk)
SKEW = 2            # attention software-pipeline depth (items)
GTOK = 2 * NT       # 394 tokens per group (2 batches)
NG = BPC // 2       # 4 groups
ITEMS_PG = 2 * KC   # 12 attention items per group


def build(qkv_bias_nonzero: bool, proj_bias_nonzero: bool):
    nc = bacc.Bacc("TRN2", target_bir_lowering=False, debug=False)

    xt = nc.dram_tensor("xt", [DIM, TOK], BF16, kind="ExternalInput")
    qkw = nc.dram_tensor("qkw", [DIM, 2 * DIM], BF16, kind="ExternalInput")
    vw = nc.dram_tensor("vw", [DIM, DIM], BF16, kind="ExternalInput")
    pw = nc.dram_tensor("pw", [DIM, DIM], BF16, kind="ExternalInput")
    ebh = nc.dram_tensor("ebh", [128, H * 2 * NT], BF16, kind="ExternalInput")
    out = nc.dram_tensor("out", [DIM, TOK], F32, kind="ExternalOutput")
    if qkv_bias_nonzero:
        qkb = nc.dram_tensor("qkb", [1, 2 * DIM], BF16, kind="ExternalInput")
        vb = nc.dram_tensor("vb", [1, DIM], BF16, kind="ExternalInput")
    if proj_bias_nonzero:
        pb = nc.dram_tensor("pb", [1, DIM], BF16, kind="ExternalInput")

    with TileContext(nc) as tc:
        with (
            tc.tile_pool(name="const", bufs=1) as constp,
            tc.tile_pool(name="vp", bufs=16) as vp,
            tc.tile_pool(name="pp", bufs=12) as pp,
            tc.tile_pool(name="rcp", bufs=4) as rcp,
            tc.tile_pool(name="obp", bufs=4) as obp,
            tc.tile_pool(name="gemm", bufs=2, space="PSUM") as gemm,
            tc.tile_pool(name="otsu", bufs=3, space="PSUM") as otsu,
            tc.tile_pool(name="sta", bufs=3, space="PSUM") as sta,
        ):
            # ---- resident constants & activations ----
            xb_s = constp.tile([128, KC, TOK], BF16, name="xb_s")
            qkw_s = constp.tile([128, KC, 2 * DIM], BF16, name="qkw_s")
            vw_s = constp.tile([128, KC, DIM], BF16, name="vw_s")
            pw_s = constp.tile([128, KC, DIM], BF16, name="pw_s")
            ebh_s = constp.tile([128, H, 2 * NT], BF16, name="ebh_s")
            ones_bf = constp.tile([128, 64], BF16, name="ones_bf")
            nc.gpsimd.memset(ones_bf[:, :], 1.0)
            wrm = constp.tile([128, 512], BF16, name="wrm")
            nc.gpsimd.memset(wrm[:, :], 0.0)
            # DMA order: group-0 x cols + first qkw halves unblock the first
            # GEMM chains ASAP; everything else streams in under compute.
            for kc in range(KC):
                nc.sync.dma_start(xb_s[:, kc, 0:GTOK],
                                  xt[kc * 128:(kc + 1) * 128, 0:GTOK])
                nc.sync.dma_start(qkw_s[:, kc, 0:DIM],
                                  qkw[kc * 128:(kc + 1) * 128, 0:DIM])
            def dma_xb_group(g):
                for kc in range(KC):
                    nc.sync.dma_start(
                        xb_s[:, kc, g * GTOK:(g + 1) * GTOK],
                        xt[kc * 128:(kc + 1) * 128, g * GTOK:(g + 1) * GTOK])
            for kc in range(KC):
                nc.sync.dma_start(qkw_s[:, kc, DIM:2 * DIM],
                                  qkw[kc * 128:(kc + 1) * 128, DIM:2 * DIM])
            dma_xb_group(1)
            for kc in range(KC):
                nc.sync.dma_start(vw_s[:, kc, :], vw[kc * 128:(kc + 1) * 128, :])
            dma_xb_group(2)
            nc.sync.dma_start(
                ebh_s[:, :, :].rearrange("p h x -> p (h x)"), ebh[:, :])
            dma_xb_group(3)
            for kc in range(KC):
                nc.sync.dma_start(pw_s[:, kc, :], pw[kc * 128:(kc + 1) * 128, :])
            # PE_HAM warmup: ~3.5us of dummy matmuls during the DMA ramp so
            # the clock gate is at 8/8 when the real GEMM chains start.
            wacc = gemm.tile([128, 512], F32, name="wacc", tag="mm")
            for _ in range(9):
                nc.tensor.matmul(wacc[:, 0:512], wrm[:, 0:128], wrm[:, 0:512],
                                 start=True, stop=True)
            wsink = constp.tile([128, 8], F32, name="wsink")
            nc.vector.tensor_copy(wsink[:, :], wacc[:, 0:8])
            qk_s = constp.tile([128, 2 * KC, TOK], BF16, name="qk_s")
            op_s = constp.tile([128, KC, TOK], BF16, name="op_s")
            if qkv_bias_nonzero:
                qkb_s = constp.tile([1, 2 * DIM], BF16, name="qkb_s")
                vb_s = constp.tile([1, DIM], BF16, name="vb_s")
                nc.sync.dma_start(qkb_s[:, :], qkb[:, :])
                nc.sync.dma_start(vb_s[:, :], vb[:, :])
            if proj_bias_nonzero:
                pb_s = constp.tile([1, DIM], BF16, name="pb_s")
                nc.sync.dma_start(pb_s[:, :], pb[:, :])
            if qkv_bias_nonzero or proj_bias_nonzero:
                ones_bfr = constp.tile([1, 512], BF16, name="ones_bfr")
                nc.gpsimd.memset(ones_bfr[:, :], 1.0)

            vt = [[None, None] for _ in range(BPC)]

            # ---- GEMM chain emitters (one PSUM acc + 6 matmuls + copy) ----
            def qk_chain(c, g):
                t0, t1 = g * GTOK, (g + 1) * GTOK
                acc = gemm.tile([128, 512], F32, name="acc_qk", tag="mm")
                for kc in range(KC):
                    nc.tensor.matmul(
                        acc[:, 0:GTOK],
                        qkw_s[:, kc, c * 128:(c + 1) * 128],
                        xb_s[:, kc, t0:t1],
                        start=(kc == 0),
                        stop=(kc == KC - 1) and not qkv_bias_nonzero,
                    )
                if qkv_bias_nonzero:
                    nc.tensor.matmul(
                        acc[:, 0:GTOK],
                        qkb_s[0:1, c * 128:(c + 1) * 128],
                        ones_bfr[0:1, 0:GTOK],
                        start=False, stop=True,
                    )
                nc.scalar.copy(qk_s[:, c, t0:t1], acc[:, 0:GTOK])

            def v_chain(b, tch, half):
                toff = b * NT + tch * 128
                tlen = 128 if tch == 0 else NT2
                n0, n1 = half * 384, (half + 1) * 384
                if half == 0:
                    vt[b][tch] = vp.tile([128, DIM], BF16, name="v_t", tag="v")
                t = vt[b][tch]
                acc = gemm.tile([128, 512], F32, name="acc_v", tag="mm")
                for kc in range(KC):
                    nc.tensor.matmul(
                        acc[0:tlen, 0:384],
                        xb_s[:, kc, toff:toff + tlen],
                        vw_s[:, kc, n0:n1],
                        start=(kc == 0),
                        stop=(kc == KC - 1) and not qkv_bias_nonzero,
                    )
                if qkv_bias_nonzero:
                    nc.tensor.matmul(
                        acc[0:tlen, 0:384],
                        ones_bfr[0:1, 0:tlen],
                        vb_s[0:1, n0:n1],
                        start=False, stop=True,
                    )
                nc.vector.tensor_copy(t[0:tlen, n0:n1], acc[0:tlen, 0:384])

            def proj_chain(c, t0, t1):
                acc = gemm.tile([128, 512], F32, name="acc_p", tag="mm")
                w = t1 - t0
                for kp in range(KC):
                    nc.tensor.matmul(
                        acc[:, 0:w],
                        pw_s[:, kp, c * 128:(c + 1) * 128],
                        op_s[:, kp, t0:t1],
                        start=(kp == 0),
                        stop=(kp == KC - 1) and not proj_bias_nonzero,
                    )
                if proj_bias_nonzero:
                    nc.tensor.matmul(
                        acc[:, 0:w],
                        pb_s[0:1, c * 128:(c + 1) * 128],
                        ones_bfr[0:1, 0:w],
                        start=False, stop=True,
                    )
                w = t1 - t0
                obt = obp.tile([128, GTOK], F32, name="obt", tag="ob")
                nc.scalar.copy(obt[:, 0:w], acc[:, 0:w])
                nc.sync.dma_start(out[c * 128:(c + 1) * 128, t0:t1],
                                  obt[:, 0:w])

            # ---- attention stages ----
            def stage_a(b, hp):
                """Scores for heads 2hp,2hp+1 (per-head PSUM banks, PE
                row-packed); strided exp per head -> P^T [128,2,197] bf16
                (j1, j2; j2 rows 69:128 don't-care); bias multiply."""
                h0 = 2 * hp
                st0 = sta.tile([128, 512], F32, name="st0", tag="sta")
                st1 = sta.tile([128, 512], F32, name="st1", tag="sta")
                q0 = qk_s[0:64, hp, b * NT:(b + 1) * NT]
                q1 = qk_s[64:128, hp, b * NT:(b + 1) * NT]
                kq = qk_s[:, KC + hp, :]
                nc.tensor.matmul(st0[:, 0:NT], kq[0:64, b * NT:b * NT + 128],
                                 q0, start=True, stop=True)
                nc.tensor.matmul(st1[:, 0:NT], kq[64:128, b * NT:b * NT + 128],
                                 q1, start=True, stop=True)
                nc.tensor.matmul(st0[0:NT2, 256:256 + NT],
                                 kq[0:64, b * NT + 128:(b + 1) * NT],
                                 q0, start=True, stop=True)
                nc.tensor.matmul(st1[0:NT2, 256:256 + NT],
                                 kq[64:128, b * NT + 128:(b + 1) * NT],
                                 q1, start=True, stop=True)
                pj0 = pp.tile([128, 2, NT], BF16, name="pj0", tag="p")
                pj1 = pp.tile([128, 2, NT], BF16, name="pj1", tag="p")
                EXP = mybir.ActivationFunctionType.Exp
                nc.scalar.activation(
                    pj0[:, :, :],
                    st0[:, 0:512].rearrange("p (c x) -> p c x", c=2)[:, :, 0:NT],
                    EXP)
                nc.scalar.activation(
                    pj1[:, :, :],
                    st1[:, 0:512].rearrange("p (c x) -> p c x", c=2)[:, :, 0:NT],
                    EXP)
                nc.vector.tensor_mul(
                    pj0[:, :, :].rearrange("p c x -> p (c x)"),
                    pj0[:, :, :].rearrange("p c x -> p (c x)"),
                    ebh_s[:, h0, :])
                nc.vector.tensor_mul(
                    pj1[:, :, :].rearrange("p c x -> p (c x)"),
                    pj1[:, :, :].rearrange("p c x -> p (c x)"),
                    ebh_s[:, h0 + 1, :])
                return pj0, pj1

            def stage_b(b, hp, pj0, pj1):
                """O^T (head pair col-packed, cols 0:197) + denominators
                (cols 256:453; h0 on partitions 0:64, h1 on 64:128) in one
                PSUM bank; single-op reciprocal + normalizing copy."""
                h0, h1 = 2 * hp, 2 * hp + 1
                bank = otsu.tile([128, 512], F32, name="bank", tag="ot")
                nc.tensor.matmul(
                    bank[0:64, 0:NT],
                    vt[b][0][:, h0 * HD:(h0 + 1) * HD],
                    pj0[:, 0, :], start=True, stop=False,
                    tile_position=(0, 0))
                nc.tensor.matmul(
                    bank[64:128, 0:NT],
                    vt[b][0][:, h1 * HD:(h1 + 1) * HD],
                    pj1[:, 0, :], start=True, stop=False,
                    tile_position=(0, 64))
                nc.tensor.matmul(
                    bank[0:64, 0:NT],
                    vt[b][1][0:NT2, h0 * HD:(h0 + 1) * HD],
                    pj0[0:NT2, 1, :], start=False, stop=True,
                    tile_position=(0, 0))
                nc.tensor.matmul(
                    bank[64:128, 0:NT],
                    vt[b][1][0:NT2, h1 * HD:(h1 + 1) * HD],
                    pj1[0:NT2, 1, :], start=False, stop=True,
                    tile_position=(0, 64))
                nc.tensor.matmul(bank[0:64, 256:256 + NT], ones_bf[:, :],
                                 pj0[:, 0, :], start=True, stop=False,
                                 tile_position=(0, 0))
                nc.tensor.matmul(bank[64:128, 256:256 + NT], ones_bf[:, :],
                                 pj1[:, 0, :], start=True, stop=False,
                                 tile_position=(0, 64))
                nc.tensor.matmul(bank[0:64, 256:256 + NT], ones_bf[0:NT2, :],
                                 pj0[0:NT2, 1, :], start=False, stop=True,
                                 tile_position=(0, 0))
                nc.tensor.matmul(bank[64:128, 256:256 + NT], ones_bf[0:NT2, :],
                                 pj1[0:NT2, 1, :], start=False, stop=True,
                                 tile_position=(0, 64))
                rc = rcp.tile([128, NT], F32, name="rc", tag="rc")
                nc.vector.reciprocal_approx_fast(
                    out=rc[:, :], in_=bank[:, 256:256 + NT])
                nc.vector.tensor_mul(
                    op_s[:, hp, b * NT:(b + 1) * NT],
                    bank[:, 0:NT], rc[:, :])

            # ---- fused schedule over 2-batch groups ----
            def group_fillers(g):
                fs = [functools.partial(qk_chain, c, g) for c in range(2 * KC)]
                for b in (2 * g, 2 * g + 1):
                    for tch in range(2):
                        for half in range(2):
                            fs.append(functools.partial(v_chain, b, tch, half))
                return fs

            # prologue: group 0 GEMMs emitted up front
            for f in group_fillers(0):
                f()

            items = [(hp, b) for b in range(BPC) for hp in range(KC)]

            def emit_proj_after(j):
                hp_j, b_j = items[j]
                if hp_j != KC - 1 or b_j % 2 != 1:
                    return
                g = b_j // 2
                for c in range(KC):
                    proj_chain(c, g * GTOK, (g + 1) * GTOK)

            pend = {}
            fill_q = []
            for i, (hp, b) in enumerate(items):
                g = b // 2
                if i % ITEMS_PG == 0 and g + 1 < NG:
                    fill_q.extend(group_fillers(g + 1))
                pend[i] = (b, hp) + tuple(stage_a(b, hp))
                j = i - SKEW
                if j >= 0:
                    stage_b(*pend.pop(j))
                    emit_proj_after(j)
                for _ in range(2):
                    if fill_q:
                        fill_q.pop(0)()
            for j in sorted(pend):
                stage_b(*pend.pop(j))
                emit_proj_after(j)

    nc.compile()
    return nc


@functools.lru_cache(maxsize=4)
def _built(qkv_bias_nonzero: bool, proj_bias_nonzero: bool):
    return build(qkv_bias_nonzero, proj_bias_nonzero)


def prepare_inputs(x, qkv_w, q_bias, v_bias, rpb_table, proj_w, proj_b, rel_index):
    """Host-side prep: shard + transpose + fold scale + gather bias table."""
    x = np.asarray(x, dtype=np.float32)
    qkv_w = np.asarray(qkv_w, dtype=np.float32)
    q_bias = np.asarray(q_bias, dtype=np.float32)
    v_bias = np.asarray(v_bias, dtype=np.float32)
    rpb_table = np.asarray(rpb_table, dtype=np.float32)
    proj_w = np.asarray(proj_w, dtype=np.float32)
    proj_b = np.asarray(proj_b, dtype=np.float32)
    rel_index = np.asarray(rel_index)

    qw = qkv_w[0:DIM] * np.float32(SCALE)   # exact: SCALE is a power of two
    qkw_h = np.ascontiguousarray(
        np.concatenate([qw, qkv_w[DIM:2 * DIM]], axis=0).T).astype(
        ml_dtypes.bfloat16)                                      # [768, 1536]
    vw_h = np.ascontiguousarray(qkv_w[2 * DIM:3 * DIM].T).astype(
        ml_dtypes.bfloat16)                                      # [768, 768]
    pw_h = np.ascontiguousarray(proj_w.T).astype(ml_dtypes.bfloat16)

    # bias[i, j, h] -> exp -> ebT[h, j, i] -> per-head [j1 | j2] blocks
    bias = rpb_table[rel_index]                                  # (197,197,12)
    ebT = np.exp(bias.astype(np.float32)).transpose(2, 1, 0)     # (12, j, i)
    ebh_f = np.ones((128, H, 2, NT), dtype=np.float32)
    ebh_f[:, :, 0, :] = ebT[:, 0:128, :].transpose(1, 0, 2)      # j1: j=0:128
    ebh_f[0:NT2, :, 1, :] = ebT[:, 128:NT, :].transpose(1, 0, 2)  # j2
    ebh_h = np.ascontiguousarray(
        ebh_f.reshape(128, H * 2 * NT)).astype(ml_dtypes.bfloat16)

    qkv_bias_nonzero = bool(q_bias.any() or v_bias.any())
    proj_bias_nonzero = bool(proj_b.any())

    in_maps = []
    for i in range(NCORES):
        xs = x[i * BPC:(i + 1) * BPC].reshape(TOK, DIM)
        m = {
            "xt": np.ascontiguousarray(xs.T).astype(ml_dtypes.bfloat16),
            "qkw": qkw_h, "vw": vw_h, "pw": pw_h,
            "ebh": ebh_h,
        }
        if qkv_bias_nonzero:
            m["qkb"] = np.ascontiguousarray(
                np.concatenate([q_bias * np.float32(SCALE),
                                np.zeros_like(q_bias)])[None, :],
                dtype=np.float32).astype(ml_dtypes.bfloat16)
            m["vb"] = np.ascontiguousarray(
                v_bias[None, :]).astype(ml_dtypes.bfloat16)
        if proj_bias_nonzero:
            m["pb"] = np.ascontiguousarray(
                proj_b[None, :], dtype=np.float32).astype(ml_dtypes.bfloat16)
        in_maps.append(m)
    return in_maps, qkv_bias_nonzero, proj_bias_nonzero


def kernel(x, qkv_w, q_bias, v_bias, rpb_table, proj_w, proj_b, rel_index):
    in_maps, qb_nz, pb_nz = prepare_inputs(
        x, qkv_w, q_bias, v_bias, rpb_table, proj_w, proj_b, rel_index)
    nc = _built(qb_nz, pb_nz)
    res = run_bass_kernel_spmd(nc, in_maps, core_ids=list(range(NCORES)))
    outs = []
    for i in range(NCORES):
        ofm = res.results[i]["out"]                  # [768, 1576]
        outs.append(ofm.T.reshape(BPC, NT, DIM))
    return np.concatenate(outs, axis=0).astype(np.float32)


# revision 13
# speedup vs baseline: 1.1972x; 1.1972x over previous
"""Trainium2 Bass kernel for windowed multi-head attention (nn_AttentionWindow).

Reference computation (B=64, N=197, DIM=768, H=12, HD=64):
    qkv  = x @ qkv_w.T + [q_bias, 0, v_bias]
    q, k, v = split(qkv);  q *= HD**-0.5
    attn = softmax(q @ k.T + rpb_table[rel_index])
    out  = (attn @ v) @ proj_w.T + proj_b
Sharding: data-parallel over batch across 8 NeuronCores (8 batches/core).

Per-core design (bf16 matmuls on TensorE, fp32 PSUM accumulation), fully
software-pipelined over 2-batch groups so TensorE never idles (keeps the
PE_HAM clock gate at 2.4 GHz):
  - x pre-transposed on host to xT [768, 1576] bf16, resident in SBUF.
    q,k computed feature-major into a resident [128, 12, 1576] bf16 tile,
    v token-major per batch ([tokens, channels] bf16, 128+69 row chunks).
  - Attention per (head-pair, batch) item: S^T[j,i] = k_h[:,j]^T q_h into
    per-head PSUM banks [128,512] (j1 @0:197, j2 @256:453), heads of the
    pair PE row-packed (concurrent). Softmax WITHOUT max subtraction
    (scores O(1): q pre-scaled by 1/8): one strided exp per head on
    ScalarE -> per-head P^T tile [128, 2, 197] bf16 (j2 rows 69:128 are
    don't-care), one contiguous DVE multiply per head against the
    precomputed exp(bias) table ebh: softmax(S+B) = expS*expB / sums.
  - O^T col-packed head pair at cols 0:197 of one PSUM bank, softmax
    denominators via ones[*,64] matmuls at cols 256:453 of the same bank
    (h0 on partitions 0:64, h1 on 64:128) -> single-op
    reciprocal_approx_fast + single-op normalizing multiply (fused into
    the PSUM->SBUF copy assembling proj's rhs op_s [128, 6, 1576] bf16).
  - The qkv/v GEMM chains of group g+1 and the projection of group g are
    interleaved between group g's attention items as dependency-free PE
    filler: attention's softmax latency chain hides under them.
  - DMA order: xb/qkw rows first (unblock the first GEMMs), vw, bias
    tables, then pw (needed only ~60us in).
"""
import sys
import functools

sys.path.insert(0, "/opt/trn_rl_repo")

import numpy as np
import ml_dtypes

import concourse.bass as bass  # noqa: E402
import concourse.bacc as bacc  # noqa: E402
import concourse.mybir as mybir  # noqa: E402
from concourse.tile import TileContext  # noqa: E402
from concourse.bass_utils import run_bass_kernel_spmd  # noqa: E402

F32 = mybir.dt.float32
BF16 = mybir.dt.bfloat16

NCORES = 8
B, NT, DIM = 64, 197, 768
H, HD = 12, 64
SCALE = HD ** -0.5  # 0.125, exact power of two -> folded into q weights
BPC = B // NCORES   # 8 batches per core
TOK = BPC * NT      # 1576 tokens per core
KC = DIM // 128     # 6
NT2 = NT - 128      # 69 (second token chunk)
SKEW = 2            # attention software-pipeline depth (items)
GTOK = 2 * NT       # 394 tokens per group (2 batches)
NG = BPC // 2       # 4 groups
ITEMS_PG = 2 * KC   # 12 attention items per group


def build(qkv_bias_nonzero: bool, proj_bias_nonzero: bool):
    nc = bacc.Bacc("TRN2", target_bir_lowering=False, debug=False)

    xt = nc.dram_tensor("xt", [DIM, TOK], BF16, kind="ExternalInput")
    qkw = nc.dram_tensor("qkw", [DIM, 2 * DIM], BF16, kind="ExternalInput")
    vw = nc.dram_tensor("vw", [DIM, DIM], BF16, kind="ExternalInput")
    pw = nc.dram_tensor("pw", [DIM, DIM], BF16, kind="ExternalInput")
    ebh = nc.dram_tensor("ebh", [128, H * 2 * NT], BF16, kind="ExternalInput")
    out = nc.dram_tensor("out", [DIM, TOK], F32, kind="ExternalOutput")
    if qkv_bias_nonzero:
        qkb = nc.dram_tensor("qkb", [1, 2 * DIM], BF16, kind="ExternalInput")
        vb = nc.dram_tensor("vb", [1, DIM], BF16, kind="ExternalInput")
    if proj_bias_nonzero:
        pb = nc.dram_tensor("pb", [1, DIM], BF16, kind="ExternalInput")

    with TileContext(nc) as tc:
        with (
            tc.tile_pool(name="const", bufs=1) as constp,
            tc.tile_pool(name="vp", bufs=16) as vp,
            tc.tile_pool(name="pp", bufs=12) as pp,
            tc.tile_pool(name="rcp", bufs=4) as rcp,
            tc.tile_pool(name="obp", bufs=4) as obp,
            tc.tile_pool(name="gemm", bufs=2, space="PSUM") as gemm,
            tc.tile_pool(name="otsu", bufs=2, space="PSUM") as otsu,
            tc.tile_pool(name="sta", bufs=4, space="PSUM") as sta,
        ):
            # ---- resident constants & activations ----
            xb_s = constp.tile([128, KC, TOK], BF16, name="xb_s")
            qkw_s = constp.tile([128, KC, 2 * DIM], BF16, name="qkw_s")
            vw_s = constp.tile([128, KC, DIM], BF16, name="vw_s")
            pw_s = constp.tile([128, KC, DIM], BF16, name="pw_s")
            ebh_s = constp.tile([128, H, 2 * NT], BF16, name="ebh_s")
            ones_bf = constp.tile([128, 64], BF16, name="ones_bf")
            nc.gpsimd.memset(ones_bf[:, :], 1.0)
            wrm = constp.tile([128, 512], BF16, name="wrm")
            nc.gpsimd.memset(wrm[:, :], 0.0)
            # DMA order: group-0 x cols + first qkw halves unblock the first
            # GEMM chains ASAP; everything else streams in under compute.
            for kc in range(KC):
                nc.sync.dma_start(xb_s[:, kc, 0:GTOK],
                                  xt[kc * 128:(kc + 1) * 128, 0:GTOK])
                nc.sync.dma_start(qkw_s[:, kc, 0:DIM],
                                  qkw[kc * 128:(kc + 1) * 128, 0:DIM])
            def dma_xb_group(g):
                for kc in range(KC):
                    nc.sync.dma_start(
                        xb_s[:, kc, g * GTOK:(g + 1) * GTOK],
                        xt[kc * 128:(kc + 1) * 128, g * GTOK:(g + 1) * GTOK])
            for kc in range(KC):
                nc.sync.dma_start(vw_s[:, kc, :], vw[kc * 128:(kc + 1) * 128, :])
            for kc in range(KC):
                nc.sync.dma_start(qkw_s[:, kc, DIM:2 * DIM],
                                  qkw[kc * 128:(kc + 1) * 128, DIM:2 * DIM])
            dma_xb_group(1)
            dma_xb_group(2)
            nc.sync.dma_start(
                ebh_s[:, :, :].rearrange("p h x -> p (h x)"), ebh[:, :])
            dma_xb_group(3)
            for kc in range(KC):
                nc.sync.dma_start(pw_s[:, kc, :], pw[kc * 128:(kc + 1) * 128, :])
            # PE_HAM warmup: ~3.5us of dummy matmuls during the DMA ramp so
            # the clock gate is at 8/8 when the real GEMM chains start.
            wacc = gemm.tile([128, 512], F32, name="wacc", tag="mm")
            for _ in range(40):
                nc.tensor.matmul(wacc[:, 0:128], wrm[:, 0:128], wrm[:, 0:128],
                                 start=True, stop=True)
            wsink = constp.tile([128, 8], F32, name="wsink")
            nc.vector.tensor_copy(wsink[:, :], wacc[:, 0:8])
            qk_s = constp.tile([128, 2 * KC, TOK], BF16, name="qk_s")
            op_s = constp.tile([128, KC, TOK], BF16, name="op_s")
            if qkv_bias_nonzero:
                qkb_s = constp.tile([1, 2 * DIM], BF16, name="qkb_s")
                vb_s = constp.tile([1, DIM], BF16, name="vb_s")
                nc.sync.dma_start(qkb_s[:, :], qkb[:, :])
                nc.sync.dma_start(vb_s[:, :], vb[:, :])
            if proj_bias_nonzero:
                pb_s = constp.tile([1, DIM], BF16, name="pb_s")
                nc.sync.dma_start(pb_s[:, :], pb[:, :])
            if qkv_bias_nonzero or proj_bias_nonzero:
                ones_bfr = constp.tile([1, 512], BF16, name="ones_bfr")
                nc.gpsimd.memset(ones_bfr[:, :], 1.0)

            vt = [[None, None] for _ in range(BPC)]

            # ---- GEMM chain emitters (one PSUM acc + 6 matmuls + copy) ----
            def qk_chain(c, g):
                t0, t1 = g * GTOK, (g + 1) * GTOK
                acc = gemm.tile([128, 512], F32, name="acc_qk", tag="mm")
                for kc in range(KC):
                    nc.tensor.matmul(
                        acc[:, 0:GTOK],
                        qkw_s[:, kc, c * 128:(c + 1) * 128],
                        xb_s[:, kc, t0:t1],
                        start=(kc == 0),
                        stop=(kc == KC - 1) and not qkv_bias_nonzero,
                    )
                if qkv_bias_nonzero:
                    nc.tensor.matmul(
                        acc[:, 0:GTOK],
                        qkb_s[0:1, c * 128:(c + 1) * 128],
                        ones_bfr[0:1, 0:GTOK],
                        start=False, stop=True,
                    )
                nc.scalar.copy(qk_s[:, c, t0:t1], acc[:, 0:GTOK])

            def v_chain(b, tch, half):
                toff = b * NT + tch * 128
                tlen = 128 if tch == 0 else NT2
                n0, n1 = half * 384, (half + 1) * 384
                if half == 0:
                    vt[b][tch] = vp.tile([128, DIM], BF16, name="v_t", tag="v")
                t = vt[b][tch]
                acc = gemm.tile([128, 512], F32, name="acc_v", tag="mm")
                for kc in range(KC):
                    nc.tensor.matmul(
                        acc[0:tlen, 0:384],
                        xb_s[:, kc, toff:toff + tlen],
                        vw_s[:, kc, n0:n1],
                        start=(kc == 0),
                        stop=(kc == KC - 1) and not qkv_bias_nonzero,
                    )
                if qkv_bias_nonzero:
                    nc.tensor.matmul(
                        acc[0:tlen, 0:384],
                        ones_bfr[0:1, 0:tlen],
                        vb_s[0:1, n0:n1],
                        start=False, stop=True,
                    )
                nc.vector.tensor_copy(t[0:tlen, n0:n1], acc[0:tlen, 0:384])

            def proj_chain(c, t0, t1):
                acc = gemm.tile([128, 512], F32, name="acc_p", tag="mm")
                w = t1 - t0
                for kp in range(KC):
                    nc.tensor.matmul(
                        acc[:, 0:w],
                        pw_s[:, kp, c * 128:(c + 1) * 128],
                        op_s[:, kp, t0:t1],
                        start=(kp == 0),
                        stop=(kp == KC - 1) and not proj_bias_nonzero,
                    )
                if proj_bias_nonzero:
                    nc.tensor.matmul(
                        acc[:, 0:w],
                        pb_s[0:1, c * 128:(c + 1) * 128],
                        ones_bfr[0:1, 0:w],
                        start=False, stop=True,
                    )
                w = t1 - t0
                obt = obp.tile([128, GTOK], F32, name="obt", tag="ob")
                nc.scalar.copy(obt[:, 0:w], acc[:, 0:w])
                nc.sync.dma_start(out[c * 128:(c + 1) * 128, t0:t1],
                                  obt[:, 0:w])

            # ---- attention stages ----
            def stage_a(b, hp):
                """Scores for heads 2hp,2hp+1 (per-head PSUM banks, PE
                row-packed); strided exp per head -> P^T [128,2,197] bf16
                (j1, j2; j2 rows 69:128 don't-care); bias multiply."""
                h0 = 2 * hp
                st0 = sta.tile([128, 512], F32, name="st0", tag="sta")
                st1 = sta.tile([128, 512], F32, name="st1", tag="sta")
                q0 = qk_s[0:64, hp, b * NT:(b + 1) * NT]
                q1 = qk_s[64:128, hp, b * NT:(b + 1) * NT]
                kq = qk_s[:, KC + hp, :]
                nc.tensor.matmul(st0[:, 0:NT], kq[0:64, b * NT:b * NT + 128],
                                 q0, start=True, stop=True)
                nc.tensor.matmul(st1[:, 0:NT], kq[64:128, b * NT:b * NT + 128],
                                 q1, start=True, stop=True)
                nc.tensor.matmul(st0[0:NT2, 256:256 + NT],
                                 kq[0:64, b * NT + 128:(b + 1) * NT],
                                 q0, start=True, stop=True)
                nc.tensor.matmul(st1[0:NT2, 256:256 + NT],
                                 kq[64:128, b * NT + 128:(b + 1) * NT],
                                 q1, start=True, stop=True)
                pj0 = pp.tile([128, 2, NT], BF16, name="pj0", tag="p")
                pj1 = pp.tile([128, 2, NT], BF16, name="pj1", tag="p")
                EXP = mybir.ActivationFunctionType.Exp
                nc.scalar.activation(
                    pj0[:, :, :],
                    st0[:, 0:512].rearrange("p (c x) -> p c x", c=2)[:, :, 0:NT],
                    EXP)
                nc.scalar.activation(
                    pj1[:, :, :],
                    st1[:, 0:512].rearrange("p (c x) -> p c x", c=2)[:, :, 0:NT],
                    EXP)
                nc.vector.tensor_mul(
                    pj0[:, :, :].rearrange("p c x -> p (c x)"),
                    pj0[:, :, :].rearrange("p c x -> p (c x)"),
                    ebh_s[:, h0, :])
                nc.vector.tensor_mul(
                    pj1[:, :, :].rearrange("p c x -> p (c x)"),
                    pj1[:, :, :].rearrange("p c x -> p (c x)"),
                    ebh_s[:, h0 + 1, :])
                return pj0, pj1

            def stage_b(b, hp, pj0, pj1):
                """O^T (head pair col-packed, cols 0:197) + denominators
                (cols 256:453; h0 on partitions 0:64, h1 on 64:128) in one
                PSUM bank; single-op reciprocal + normalizing copy."""
                h0, h1 = 2 * hp, 2 * hp + 1
                bank = otsu.tile([128, 512], F32, name="bank", tag="ot")
                nc.tensor.matmul(
                    bank[0:64, 0:NT],
                    vt[b][0][:, h0 * HD:(h0 + 1) * HD],
                    pj0[:, 0, :], start=True, stop=False,
                    tile_position=(0, 0))
                nc.tensor.matmul(
                    bank[64:128, 0:NT],
                    vt[b][0][:, h1 * HD:(h1 + 1) * HD],
                    pj1[:, 0, :], start=True, stop=False,
                    tile_position=(0, 64))
                nc.tensor.matmul(
                    bank[0:64, 0:NT],
                    vt[b][1][0:NT2, h0 * HD:(h0 + 1) * HD],
                    pj0[0:NT2, 1, :], start=False, stop=True,
                    tile_position=(0, 0))
                nc.tensor.matmul(
                    bank[64:128, 0:NT],
                    vt[b][1][0:NT2, h1 * HD:(h1 + 1) * HD],
                    pj1[0:NT2, 1, :], start=False, stop=True,
                    tile_position=(0, 64))
                nc.tensor.matmul(bank[0:64, 256:256 + NT], ones_bf[:, :],
                                 pj0[:, 0, :], start=True, stop=False,
                                 tile_position=(0, 0))
                nc.tensor.matmul(bank[64:128, 256:256 + NT], ones_bf[:, :],
                                 pj1[:, 0, :], start=True, stop=False,
                                 tile_position=(0, 64))
                nc.tensor.matmul(bank[0:64, 256:256 + NT], ones_bf[0:NT2, :],
                                 pj0[0:NT2, 1, :], start=False, stop=True,
                                 tile_position=(0, 0))
                nc.tensor.matmul(bank[64:128, 256:256 + NT], ones_bf[0:NT2, :],
                                 pj1[0:NT2, 1, :], start=False, stop=True,
                                 tile_position=(0, 64))
                rc = rcp.tile([128, NT], F32, name="rc", tag="rc")
                nc.vector.reciprocal_approx_fast(
                    out=rc[:, :], in_=bank[:, 256:256 + NT])
                nc.vector.tensor_mul(
                    op_s[:, hp, b * NT:(b + 1) * NT],
                    bank[:, 0:NT], rc[:, :])

            # ---- fused schedule over 2-batch groups ----
            def group_fillers(g, split=False):
                qk1 = [functools.partial(qk_chain, c, g) for c in range(KC)]
                qk2 = [functools.partial(qk_chain, c, g)
                       for c in range(KC, 2 * KC)]
                vs = []
                for b in (2 * g, 2 * g + 1):
                    for tch in range(2):
                        for half in range(2):
                            vs.append(functools.partial(v_chain, b, tch, half))
                return qk1 + vs + qk2 if split else qk1 + qk2 + vs

            # prologue: group 0 GEMMs emitted up front, ordered to match
            # DMA arrival (q chunks, v, then k chunks)
            for f in group_fillers(0, split=True):
                f()

            items = [(hp, b) for b in range(BPC) for hp in range(KC)]

            def emit_proj_after(j):
                hp_j, b_j = items[j]
                if hp_j != KC - 1 or b_j % 2 != 1:
                    return
                g = b_j // 2
                for c in range(KC):
                    proj_chain(c, g * GTOK, (g + 1) * GTOK)

            pend = {}
            fill_q = []
            for i, (hp, b) in enumerate(items):
                g = b // 2
                if i % ITEMS_PG == 0 and g + 1 < NG:
                    fill_q.extend(group_fillers(g + 1))
                pend[i] = (b, hp) + tuple(stage_a(b, hp))
                j = i - SKEW
                if j >= 0:
                    stage_b(*pend.pop(j))
                    emit_proj_after(j)
                for _ in range(2):
                    if fill_q:
                        fill_q.pop(0)()
            for j in sorted(pend):
                stage_b(*pend.pop(j))
                emit_proj_after(j)

    nc.compile()
    return nc


@functools.lru_cache(maxsize=4)
def _built(qkv_bias_nonzero: bool, proj_bias_nonzero: bool):
    return build(qkv_bias_nonzero, proj_bias_nonzero)


def prepare_inputs(x, qkv_w, q_bias, v_bias, rpb_table, proj_w, proj_b, rel_index):
    """Host-side prep: shard + transpose + fold scale + gather bias table."""
    x = np.asarray(x, dtype=np.float32)
    qkv_w = np.asarray(qkv_w, dtype=np.float32)
    q_bias = np.asarray(q_bias, dtype=np.float32)
    v_bias = np.asarray(v_bias, dtype=np.float32)
    rpb_table = np.asarray(rpb_table, dtype=np.float32)
    proj_w = np.asarray(proj_w, dtype=np.float32)
    proj_b = np.asarray(proj_b, dtype=np.float32)
    rel_index = np.asarray(rel_index)

    qw = qkv_w[0:DIM] * np.float32(SCALE)   # exact: SCALE is a power of two
    qkw_h = np.ascontiguousarray(
        np.concatenate([qw, qkv_w[DIM:2 * DIM]], axis=0).T).astype(
        ml_dtypes.bfloat16)                                      # [768, 1536]
    vw_h = np.ascontiguousarray(qkv_w[2 * DIM:3 * DIM].T).astype(
        ml_dtypes.bfloat16)                                      # [768, 768]
    pw_h = np.ascontiguousarray(proj_w.T).astype(ml_dtypes.bfloat16)

    # bias[i, j, h] -> exp -> ebT[h, j, i] -> per-head [j1 | j2] blocks
    bias = rpb_table[rel_index]                                  # (197,197,12)
    ebT = np.exp(bias.astype(np.float32)).transpose(2, 1, 0)     # (12, j, i)
    ebh_f = np.ones((128, H, 2, NT), dtype=np.float32)
    ebh_f[:, :, 0, :] = ebT[:, 0:128, :].transpose(1, 0, 2)      # j1: j=0:128
    ebh_f[0:NT2, :, 1, :] = ebT[:, 128:NT, :].transpose(1, 0, 2)  # j2
    ebh_h = np.ascontiguousarray(
        ebh_f.reshape(128, H * 2 * NT)).astype(ml_dtypes.bfloat16)

    qkv_bias_nonzero = bool(q_bias.any() or v_bias.any())
    proj_bias_nonzero = bool(proj_b.any())

    in_maps = []
    for i in range(NCORES):
        xs = x[i * BPC:(i + 1) * BPC].reshape(TOK, DIM)
        m = {
            "xt": np.ascontiguousarray(xs.T).astype(ml_dtypes.bfloat16),
            "qkw": qkw_h, "vw": vw_h, "pw": pw_h,
            "ebh": ebh_h,
        }
        if qkv_bias_nonzero:
            m["qkb"] = np.ascontiguousarray(
                np.concatenate([q_bias * np.float32(SCALE),
                                np.zeros_like(q_bias)])[None, :],
                dtype=np.float32).astype(ml_dtypes.bfloat16)
            m["vb"] = np.ascontiguousarray(
                v_bias[None, :]).astype(ml_dtypes.bfloat16)
        if proj_bias_nonzero:
            m["pb"] = np.ascontiguousarray(
                proj_b[None, :], dtype=np.float32).astype(ml_dtypes.bfloat16)
        in_maps.append(m)
    return in_maps, qkv_bias_nonzero, proj_bias_nonzero


def kernel(x, qkv_w, q_bias, v_bias, rpb_table, proj_w, proj_b, rel_index):
    in_maps, qb_nz, pb_nz = prepare_inputs(
        x, qkv_w, q_bias, v_bias, rpb_table, proj_w, proj_b, rel_index)
    nc = _built(qb_nz, pb_nz)
    res = run_bass_kernel_spmd(nc, in_maps, core_ids=list(range(NCORES)))
    outs = []
    for i in range(NCORES):
        ofm = res.results[i]["out"]                  # [768, 1576]
        outs.append(ofm.T.reshape(BPC, NT, DIM))
    return np.concatenate(outs, axis=0).astype(np.float32)
